# revision 1
# baseline (speedup 1.0000x reference)
"""Trainium2 Bass kernel for ContrastiveNet loss (v2: fp8 DoubleRow).

Algorithm (per core k of 8, SPMD):
  - host: x cast to fp8e4 (e4m3, +-240), rolled so core k's 512 anchor rows sit
    at columns 0..511, laid out [128, 16, B] for DoubleRow k-pairing.
  - device:
      load y (fp8), squares -> sq (fp8, ACT/DVE split) during load,
      colsum-of-squares via DoubleRow matmul vs ones -> norm^2 replicated
      across partitions in PSUM, Rsqrt(0.1*n2) -> invw_bc [128,B] bf16,
      row scales invwT[p,mt] = diag pick of invw_bc via identity mask,
      gram G = Xblk @ X.T in fp8 DoubleRow (PE, 0.5 cyc/col),
      sim = G * invwT * invw_bc (DVE stt -> bf16),
      per-pair logit gather via gpsimd.local_scatter chains,
      exp/logsumexp (no slot mask needed: invalid pairs zeroed by pairmask),
      per-core partial sum -> [1,1].
  - host: sum 8 partials / P.
"""
import os
import sys
import numpy as np
import ml_dtypes

try:
    import concourse  # noqa: F401
except ImportError:
    sys.path.insert(0, "/opt/trn_rl_repo")

from contextlib import ExitStack

import concourse.bass as bass
import concourse.tile as tile
from concourse import bacc, mybir
from concourse._compat import with_exitstack
from concourse.bass_utils import run_bass_kernel_spmd

BF16 = ml_dtypes.bfloat16
FP8 = ml_dtypes.float8_e4m3
F32 = mybir.dt.float32
DBF = mybir.dt.bfloat16
F8 = mybir.dt.float8e4
I16 = mybir.dt.int16

B, D, J = 4096, 2048, 11
NCORES, RPC, NT, NKT, NKP = 8, 512, 4, 16, 8
TEMP = 0.1
AF = mybir.ActivationFunctionType
ALU = mybir.AluOpType
AX = mybir.AxisListType
DR = mybir.MatmulPerfMode.DoubleRow


# ---------------------------------------------------------------- host prep
def build_plan(anchor_idx, pos_idx, neg_idx):
    """Scatter planes; plane0 column indices are per-core ROTATED by -512k."""
    r = anchor_idx.astype(np.int64)
    cols = np.concatenate([pos_idx[:, None], neg_idx], axis=1).astype(np.int64)
    P = r.shape[0]

    order = np.argsort(r, kind="stable")
    r_sorted = r[order]
    first = np.r_[True, r_sorted[1:] != r_sorted[:-1]]
    gid = np.cumsum(first) - 1
    rank_sorted = np.arange(P) - np.flatnonzero(first)[gid]
    srank = np.empty(P, np.int64)
    srank[order] = rank_sorted
    n_per_row = np.bincount(r, minlength=B)
    SP = int(max(n_per_row.max(), 1))
    NE = SP * J + (SP * J) % 2
    assert NE * 32 < 2**16

    er = np.repeat(r, J)
    ec = cols.ravel()
    eslot = np.repeat(srank, J) * J + np.tile(np.arange(J), P)
    key = er * B + ec
    o2 = np.argsort(key, kind="stable")
    k_sorted = key[o2]
    first2 = np.r_[True, k_sorted[1:] != k_sorted[:-1]]
    gid2 = np.cumsum(first2) - 1
    occ_sorted = np.arange(P * J) - np.flatnonzero(first2)[gid2]
    occ = np.empty(P * J, np.int64)
    occ[o2] = occ_sorted
    L = int(occ.max())

    eslot_sorted = eslot[o2]
    prev_slot_sorted = np.empty(P * J, np.int64)
    prev_slot_sorted[0] = -1
    prev_slot_sorted[1:] = eslot_sorted[:-1]
    prev_slot = np.empty(P * J, np.int64)
    prev_slot[o2] = prev_slot_sorted

    core = er // RPC
    t = (er % RPC) // 128
    pp = er % 128
    ec_rot = (ec - core * RPC) % B  # per-core rotated column index

    plane0 = np.full((NCORES, NT, 128, B), -1, np.int16)
    m0 = occ == 0
    plane0[core[m0], t[m0], pp[m0], ec_rot[m0]] = eslot[m0].astype(np.int16)

    planes = []
    for q in range(1, L + 1):
        pl = np.full((NCORES, NT, 128, NE), -1, np.int16)
        mq = occ == q
        pl[core[mq], t[mq], pp[mq], prev_slot[mq]] = eslot[mq].astype(np.int16)
        planes.append(pl)

    nmat = n_per_row.reshape(NCORES, NT, 128)
    pairmask = (np.arange(SP)[None, None, None, :] < nmat[..., None]).astype(BF16)
    return dict(plane0=plane0, planes=planes, pairmask=pairmask, SP=SP, NE=NE, L=L)


# ------------------------------------------------------------- device kernel
@with_exitstack
def _build(ctx: ExitStack, tc: "tile.TileContext", io: dict, SP: int, NE: int, L: int):
    nc = tc.nc
    y_d, ident_d, pl0_d, pm_d, out_d = (
        io["y8"], io["ident"], io["plane0"], io["pm"], io["out"])
    plq_d = [io[f"plane{q}"] for q in range(1, L + 1)]
    NB = B // 128           # 32 column blocks
    BPB = 8                 # blocks per psum batch

    consts = ctx.enter_context(tc.tile_pool(name="consts", bufs=1))
    ones_f32c = consts.tile([128, 1], F32, tag="ones_f32c")
    nc.vector.memset(ones_f32c[:], 1.0)
    ident = consts.tile([128, 128], DBF, tag="ident")
    nc.sync.dma_start(ident[:], ident_d[:])

    ypool = ctx.enter_context(tc.tile_pool(name="y", bufs=1))
    y = ypool.tile([128, NKT, B], F8, tag="y", name="y")

    npool = ctx.enter_context(tc.tile_pool(name="norms", bufs=1))
    nrm2 = npool.tile([128, NB], F32, tag="nrm2")
    invw_col = npool.tile([128, NB], F32, tag="invw_col")
    invw_cb = npool.tile([128, NB], DBF, tag="invw_cb")
    invw_row = npool.tile([1, B], DBF, tag="invw_row")
    invw_bc = npool.tile([128, B], DBF, tag="invw_bc")

    lpool = ctx.enter_context(tc.tile_pool(name="loss", bufs=1))
    denall = lpool.tile([128, NT * SP], F32, tag="denall")
    l0all = lpool.tile([128, NT * SP], DBF, tag="l0all")
    pmall = lpool.tile([128, NT, SP], DBF, tag="pmall")

    # ---- x load (SP queue); pm + first planes also SP
    for kp in range(NKP):
        nc.sync.dma_start(y[:, 2 * kp:2 * kp + 2, :], y_d[:, 2 * kp:2 * kp + 2, :])

    # ---- norms from 32 block-diagonal self-grams (DoubleRow, no squares)
    with tc.tile_pool(name="bdg", bufs=2, space="PSUM") as bdg:
        bt = {}
        for bat in range(2):
            bt[bat] = bdg.tile([128, BPB * 128], F32, tag="bdg", name=f"bdg{bat}")
        for kp in range(NKP):
            for bat in range(2):
                for b in range(BPB):
                    blk = bat * BPB + b
                    nc.tensor.matmul(
                        bt[bat][:, b * 128:(b + 1) * 128],
                        lhsT=y[:, 2 * kp:2 * kp + 2, blk * 128:(blk + 1) * 128],
                        rhs=y[:, 2 * kp:2 * kp + 2, blk * 128:(blk + 1) * 128],
                        start=(kp == 0), stop=(kp == NKP - 1),
                        perf_mode=DR,
                    )
        scrapd = npool.tile([128, 128], DBF, tag="scrapd")
        for bat in range(4):
            if bat >= 2:
                bt[bat] = bdg.tile([128, BPB * 128], F32, tag="bdg", name=f"bdg{bat}")
                for kp in range(NKP):
                    for b in range(BPB):
                        blk = bat * BPB + b
                        nc.tensor.matmul(
                            bt[bat][:, b * 128:(b + 1) * 128],
                            lhsT=y[:, 2 * kp:2 * kp + 2, blk * 128:(blk + 1) * 128],
                            rhs=y[:, 2 * kp:2 * kp + 2, blk * 128:(blk + 1) * 128],
                            start=(kp == 0), stop=(kp == NKP - 1),
                            perf_mode=DR,
                        )
            for b in range(BPB):
                blk = bat * BPB + b
                nc.vector.scalar_tensor_tensor(
                    scrapd[:], bt[bat][:, b * 128:(b + 1) * 128], 1.0, ident[:],
                    ALU.mult, ALU.mult, accum_out=nrm2[:, blk:blk + 1])

        # invw_col = sqrt((1/TEMP)/nrm2); bf16 copy; transpose -> row layout
        nc.vector.reciprocal(invw_col[:], nrm2[:])
        nc.scalar.activation(invw_col[:], invw_col[:], AF.Sqrt, scale=1.0 / TEMP)
        nc.vector.tensor_copy(invw_cb[:], invw_col[:])
        ps_t = bdg.tile([NB, 128], DBF, tag="ps_t")
        nc.tensor.transpose(ps_t[:], invw_cb[:], ident[:])
        row_st = npool.tile([NB, 128], DBF, tag="row_st")
        nc.scalar.copy(row_st[:], ps_t[:])
        nc.sync.dma_start(invw_row[:], row_st[:])

    # pm for all tiles + plane prefetch for mt0 (SP queue)
    for t in range(NT):
        nc.sync.dma_start(pmall[:, t, :], pm_d[t])

    nc.gpsimd.partition_broadcast(invw_bc[:, 0:2048], invw_row[0:1, 0:2048])
    nc.gpsimd.partition_broadcast(invw_bc[:, 2048:B], invw_row[0:1, 2048:B])

    with tc.tile_pool(name="gpsum", bufs=2, space="PSUM") as gpsum, \
         tc.tile_pool(name="gbf", bufs=2) as gbfpool, \
         tc.tile_pool(name="pl0", bufs=2) as pl0pool, \
         tc.tile_pool(name="plq", bufs=2) as plqpool, \
         tc.tile_pool(name="slots", bufs=2) as slpool, \
         tc.tile_pool(name="elb", bufs=2) as elpool:

        # prefetch mt0 planes (SP queue)
        pl0_t = {0: pl0pool.tile([128, B], I16, tag="pl0", name="pl0_0")}
        nc.sync.dma_start(pl0_t[0][:], pl0_d[0])
        plq_t = {(0, q): plqpool.tile([128, NE], I16, tag=f"plq{q}", name=f"plq_0_{q}")
                 for q in range(1, L + 1)}
        for q in range(1, L + 1):
            nc.sync.dma_start(plq_t[(0, q)][:], plq_d[q - 1][0])

        # ---- per row-tile: gram (DoubleRow), normalize, scatter, partial loss
        for mt in range(NT):
            gbf = gbfpool.tile([128, B], DBF, tag="gbf")
            for half in range(2):
                gps = gpsum.tile([128, 2048], F32, tag="gram")
                for kp in range(NKP):
                    for chk in range(4):
                        c0 = half * 2048 + chk * 512
                        nc.tensor.matmul(
                            gps[:, chk * 512:(chk + 1) * 512],
                            lhsT=y[:, 2 * kp:2 * kp + 2, mt * 128:(mt + 1) * 128],
                            rhs=y[:, 2 * kp:2 * kp + 2, c0:c0 + 512],
                            start=(kp == 0), stop=(kp == NKP - 1),
                            perf_mode=DR,
                        )
                nc.vector.scalar_tensor_tensor(
                    gbf[:, half * 2048:(half + 1) * 2048], gps[:],
                    invw_col[:, mt:mt + 1],
                    invw_bc[:, half * 2048:(half + 1) * 2048],
                    ALU.mult, ALU.mult,
                )

            # prefetch next tile's planes (SP queue)
            if mt + 1 < NT:
                pl0_t[mt + 1] = pl0pool.tile([128, B], I16, tag="pl0", name=f"pl0_{mt+1}")
                nc.sync.dma_start(pl0_t[mt + 1][:], pl0_d[mt + 1])
                for q in range(1, L + 1):
                    plq_t[(mt + 1, q)] = plqpool.tile([128, NE], I16, tag=f"plq{q}", name=f"plq_{mt+1}_{q}")
                    nc.sync.dma_start(plq_t[(mt + 1, q)][:], plq_d[q - 1][mt + 1])

            # scatter chain
            s_lv = slpool.tile([128, NE], DBF, tag="slv0")
            nc.gpsimd.local_scatter(s_lv[:], gbf[:], pl0_t[mt][:], 128, NE, B)
            s_all = slpool.tile([128, NE], DBF, tag="s_all")
            nc.vector.tensor_copy(s_all[:], s_lv[:])
            for q in range(1, L + 1):
                s_nx = slpool.tile([128, NE], DBF, tag=f"slv{q % 2 + 1}")
                nc.gpsimd.local_scatter(s_nx[:], s_lv[:], plq_t[(mt, q)][:],
                                        128, NE, NE)
                nc.vector.tensor_tensor(s_all[:], s_all[:], s_nx[:], ALU.add)
                s_lv = s_nx

            # exp + per-pair denominators; Ln batched after the loop
            ebuf = elpool.tile([128, NE], F32, tag="ebuf")
            nc.scalar.activation(ebuf[:], s_all[:], AF.Exp)
            e3 = ebuf[:, 0:SP * J].rearrange("p (s j) -> p s j", j=J)
            nc.vector.tensor_reduce(denall[:, mt * SP:(mt + 1) * SP], e3,
                                    AX.X, ALU.add)
            l0 = s_all[:, 0:SP * J].rearrange("p (s j) -> p s j", j=J)[:, :, 0]
            nc.vector.tensor_copy(l0all[:, mt * SP:(mt + 1) * SP], l0)

    # ---- batched logsumexp tail + total
    with tc.tile_pool(name="p5psum", bufs=1, space="PSUM") as p5psum:
        lnd = lpool.tile([128, NT * SP], F32, tag="lnd")
        nc.scalar.activation(lnd[:], denall[:], AF.Ln)
        diff = lpool.tile([128, NT * SP], F32, tag="diff")
        nc.vector.scalar_tensor_tensor(diff[:], l0all[:], -1.0, lnd[:],
                                       ALU.mult, ALU.add)
        scrap = lpool.tile([128, NT * SP], F32, tag="scrap")
        acc1 = lpool.tile([128, 1], F32, tag="acc1")
        nc.vector.scalar_tensor_tensor(
            scrap[:], diff[:], 1.0,
            pmall[:].rearrange("p t s -> p (t s)"), ALU.mult, ALU.mult,
            accum_out=acc1[:],
        )
        ps = p5psum.tile([1, 1], F32, tag="ps_out")
        nc.tensor.matmul(ps[:], lhsT=acc1[:], rhs=ones_f32c[:, 0:1],
                         start=True, stop=True)
        res = lpool.tile([1, 1], F32, tag="res")
        nc.scalar.copy(res[:], ps[:])
        nc.sync.dma_start(out_d[:], res[:])


def build_nc(SP, NE, L, enable_asserts=False):
    nc = bacc.Bacc("TRN2", target_bir_lowering=False, debug=False,
                   enable_asserts=enable_asserts, num_devices=NCORES)
    io = {
        "y8": nc.dram_tensor("y8", [128, NKT, B], F8, kind="ExternalInput").ap(),
        "ident": nc.dram_tensor("ident", [128, 128], DBF, kind="ExternalInput").ap(),
        "plane0": nc.dram_tensor("plane0", [NT, 128, B], I16, kind="ExternalInput").ap(),
        "pm": nc.dram_tensor("pm", [NT, 128, SP], DBF, kind="ExternalInput").ap(),
        "out": nc.dram_tensor("out", [1, 1], F32, kind="ExternalOutput").ap(),
    }
    for q in range(1, L + 1):
        io[f"plane{q}"] = nc.dram_tensor(
            f"plane{q}", [NT, 128, NE], I16, kind="ExternalInput").ap()
    with tile.TileContext(nc) as tc:
        _build(tc, io, SP, NE, L)
    nc.compile()
    return nc


def make_in_maps(x, plan):
    x8 = np.clip(np.asarray(x, np.float32), -240.0, 240.0).astype(FP8)
    ident = np.eye(128, dtype=BF16)
    in_maps = []
    for k in range(NCORES):
        xr = np.roll(x8, -RPC * k, axis=0)          # [B, D]
        y8 = np.ascontiguousarray(
            xr.T.reshape(NKT, 128, B).transpose(1, 0, 2))  # [128, NKT, B]
        m = {
            "y8": y8,
            "ident": ident,
            "plane0": plan["plane0"][k],
            "pm": plan["pairmask"][k],
        }
        for q in range(1, plan["L"] + 1):
            m[f"plane{q}"] = plan["planes"][q - 1][k]
        in_maps.append(m)
    return in_maps


def kernel(**inputs):
    x = np.asarray(inputs["x"], np.float32)
    anchor_idx = np.asarray(inputs["anchor_idx"])
    pos_idx = np.asarray(inputs["pos_idx"])
    neg_idx = np.asarray(inputs["neg_idx"])
    P = anchor_idx.shape[0]

    plan = build_plan(anchor_idx, pos_idx, neg_idx)
    nc = build_nc(plan["SP"], plan["NE"], plan["L"])
    in_maps = make_in_maps(x, plan)
    res = run_bass_kernel_spmd(nc, in_maps, list(range(NCORES)))
    total = sum(float(res.results[k]["out"][0, 0]) for k in range(NCORES))
    return np.float32(total / P)



# revision 10
# speedup vs baseline: 1.3408x; 1.3408x over previous
"""Trainium2 Bass kernel for ContrastiveNet loss (v3: host-normalized rows,
chunk-major gram pipeline, device-generated scatter planes).

Algorithm (per core k of 8, SPMD):
  - host: xn = x / ||x|| * S (S=32), cast fp8e4 (so sim = G / (S^2*TEMP) with
    G the raw fp8 gram; no on-device normalization at all). Rows rolled so
    core k's 512 anchor rows sit at rotated columns 0..511. y laid out
    COLUMN-CHUNK-major: [8 chunks][128 part][16 kt][512 cols] so each chunk's
    gram (all 4 row-tiles x 8 kp DoubleRow matmuls) runs as soon as the chunk
    lands -> gram fully overlaps the 23us HBM load.
  - device, per column chunk c: 4x8 fp8 DR matmuls into [128,512] PSUM tiles
    (8 banks: 4 tiles x double buffer), drained to fp16 SBUF alternately by
    DVE/ACT.
  - gather: per (tile, quarter=1024 cols): gpsimd local_scatter using a plane
    (col->slot map) GENERATED ON DEVICE from compact occ0 entry lists
    (col,slot+1) via a small scatter + DVE -1 bias (unset cols -> -1).
    Duplicate (row,col) refs are fixed by parallel passes: pass1 (full NE
    scan) and narrow passes q>=2 (sources packed at low slots via pair
    ranking), all reading the summed occ0 slots.
  - loss: exp(scale*logits) on ACT, per-pair den reduce + masked
    (ln den - scale*l0) accumulation, single-partial [1,1] out per core.
  - host: sum 8 partials / P.
"""
import os
import sys
import numpy as np
import ml_dtypes

try:
    import concourse  # noqa: F401
except ImportError:
    sys.path.insert(0, "/opt/trn_rl_repo")

from contextlib import ExitStack

import concourse.bass as bass
import concourse.tile as tile
from concourse import bacc, mybir
from concourse._compat import with_exitstack
from concourse.bass_utils import run_bass_kernel_spmd

F16 = np.float16
FP8 = ml_dtypes.float8_e4m3
F32 = mybir.dt.float32
DF16 = mybir.dt.float16
F8 = mybir.dt.float8e4
I16 = mybir.dt.int16

B, D, J = 4096, 2048, 11
NCORES, RPC, NT, NKP = 8, 512, 4, 8
NCH, CW = 8, 512          # column chunks of the gram (per core)
NQ, QW = 4, 1024          # scatter quarters
TEMP = 0.1
S = 32.0                  # host pre-scale of normalized rows
KSC = 1.0 / (S * S * TEMP)
AF = mybir.ActivationFunctionType
ALU = mybir.AluOpType
AX = mybir.AxisListType
DR = mybir.MatmulPerfMode.DoubleRow


def _even(n):
    return n + (n % 2)


# ---------------------------------------------------------------- host prep
def build_plan(anchor_idx, pos_idx, neg_idx):
    r = anchor_idx.astype(np.int64)
    cols = np.concatenate([pos_idx[:, None], neg_idx], axis=1).astype(np.int64)
    P = r.shape[0]

    # ---- duplicate groups over (row, col)
    er = np.repeat(r, J)
    ec = cols.ravel()
    pair_of = np.repeat(np.arange(P), J)
    key = er * B + ec
    o2 = np.argsort(key, kind="stable")
    k_sorted = key[o2]
    first2 = np.r_[True, k_sorted[1:] != k_sorted[:-1]]
    gid_sorted = np.cumsum(first2) - 1
    NG = int(gid_sorted[-1]) + 1
    gid = np.empty(P * J, np.int64)
    gid[o2] = gid_sorted
    occ_sorted = np.arange(P * J) - np.flatnonzero(first2)[gid_sorted]
    occ = np.empty(P * J, np.int64)
    occ[o2] = occ_sorted
    gsz_g = np.bincount(gid_sorted, minlength=NG)       # per-group size
    gsz = gsz_g[gid]                                     # per-entry
    MAXO = int(occ.max())                                # passes 1..MAXO
    # group source entry = first occurrence (smallest (pair, j)); its pair
    e0 = o2[np.flatnonzero(first2)]                      # entry idx per group
    src_pair_g = pair_of[e0]

    # ---- pair ranking per row: pairs sourcing big dup groups first
    sev = np.zeros(P, np.int64)
    big = gsz_g >= 3
    np.maximum.at(sev, src_pair_g[big], gsz_g[big])
    order_p = np.lexsort((np.arange(P), -sev, r))
    r_sp = r[order_p]
    firstp = np.r_[True, r_sp[1:] != r_sp[:-1]]
    gidp = np.cumsum(firstp) - 1
    rank_sorted = np.arange(P) - np.flatnonzero(firstp)[gidp]
    srank = np.empty(P, np.int64)
    srank[order_p] = rank_sorted

    n_per_row = np.bincount(r, minlength=B)
    SP = int(max(n_per_row.max(), 1))
    NE = _even(SP * J)
    assert NE * 32 < 2**16

    eslot = srank[pair_of] * J + np.tile(np.arange(J), P)   # slot per entry

    # ---- pass planes q = 1..MAXO: source occ0 slot -> occ q slot
    slot0_g = np.empty(NG, np.int64)
    m0 = occ == 0
    slot0_g[gid[m0]] = eslot[m0]
    core = er // RPC
    t = (er % RPC) // 128
    pp = er % 128

    # scan widths: pass 1 full NE; pass q>=2 sources are occ0 slots of
    # groups of size >= q+1, whose pairs were ranked < (#sev>=q+1 pairs).
    W = {1: NE}
    for q in range(2, MAXO + 1):
        cnt = np.bincount(r[sev >= q + 1], minlength=B) if (sev >= q + 1).any() \
            else np.zeros(B, np.int64)
        W[q] = _even(min(int(cnt.max()) * J + 2, NE)) if cnt.max() > 0 else 2
    planes = {}
    for q in range(1, MAXO + 1):
        pl = np.full((NCORES, NT, 128, W[q]), -1, np.int16)
        mq = occ == q
        src = slot0_g[gid[mq]]
        assert (src < W[q]).all(), f"pass {q} source slot >= W"
        pl[core[mq], t[mq], pp[mq], src] = eslot[mq].astype(np.int16)
        planes[q] = pl

    # ---- occ0 entry lists per (core, tile, partition, quarter)
    ec_rot = (ec - core * RPC) % B
    qtr = ec_rot // QW
    c_loc = ec_rot % QW
    # position within each (row, quarter) bucket
    bkey = er * 4 + qtr
    bo = np.argsort(bkey[m0], kind="stable")
    bk_sorted = bkey[m0][bo]
    firstb = np.r_[True, bk_sorted[1:] != bk_sorted[:-1]]
    gb = np.cumsum(firstb) - 1
    pos_sorted = np.arange(m0.sum()) - np.flatnonzero(firstb)[gb]
    bpos = np.empty(m0.sum(), np.int64)
    bpos[bo] = pos_sorted
    NEQ = _even(int(pos_sorted.max()) + 1)
    # lists[core][tile] layout: [128, NQ, 2, NEQ]; [:, q, 0] = local col
    # (-1 pad), [:, q, 1] = slot + 1
    lists = np.full((NCORES, NT, 128, NQ, 2, NEQ), -1, np.int16)
    lists[:, :, :, :, 1, :] = 0
    ce, te, pe_, qe = core[m0], t[m0], pp[m0], qtr[m0]
    lists[ce, te, pe_, qe, 0, bpos] = c_loc[m0].astype(np.int16)
    lists[ce, te, pe_, qe, 1, bpos] = (eslot[m0] + 1).astype(np.int16)

    nmat = n_per_row.reshape(NCORES, NT, 128)
    pairmask = (np.arange(SP)[None, None, None, :] < nmat[..., None]).astype(F16)
    return dict(lists=lists, planes=planes, pairmask=pairmask,
                SP=SP, NE=NE, NEQ=NEQ, MAXO=MAXO, W=W)


# ------------------------------------------------------------- device kernel
@with_exitstack
def _build(ctx: ExitStack, tc: "tile.TileContext", io: dict, SP: int, NE: int,
           NEQ: int, MAXO: int, W: dict):
    nc = tc.nc
    y_d, li_d, pm_d, out_d = io["y8"], io["lists"], io["pm"], io["out"]
    pq_d = {q: io[f"pass{q}"] for q in range(1, MAXO + 1)}

    consts = ctx.enter_context(tc.tile_pool(name="consts", bufs=1))
    ones_f32c = consts.tile([128, 1], F32, tag="ones_f32c")
    nc.vector.memset(ones_f32c[:], 1.0)

    ypool = ctx.enter_context(tc.tile_pool(name="y", bufs=1))
    y = ypool.tile([128, NCH, 2 * NKP, CW], F8, tag="y", name="y")

    gpool = ctx.enter_context(tc.tile_pool(name="gbf", bufs=1))
    gbf = {tt: gpool.tile([128, B], DF16, tag=f"gbf{tt}", name=f"gbf{tt}")
           for tt in range(NT)}
    plpool = ctx.enter_context(tc.tile_pool(name="plane", bufs=1))
    plane = {tt: plpool.tile([128, B], I16, tag=f"plane{tt}", name=f"plane{tt}")
             for tt in range(NT)}
    lipool = ctx.enter_context(tc.tile_pool(name="lists", bufs=2))
    li = {tt: lipool.tile([128, NQ, 2, NEQ], I16, tag="li", name=f"li{tt}")
          for tt in range(NT)}
    pqpool = ctx.enter_context(tc.tile_pool(name="passes", bufs=1))
    pq = {(tt, q): pqpool.tile([128, W[q]], I16, tag=f"pq{tt}_{q}",
                               name=f"pq{tt}_{q}")
          for tt in range(NT) for q in range(1, MAXO + 1)}

    lpool = ctx.enter_context(tc.tile_pool(name="loss", bufs=1))
    denall = lpool.tile([128, NT * SP], F32, tag="denall")
    l0all = lpool.tile([128, NT * SP], DF16, tag="l0all")
    pmall = lpool.tile([128, NT, SP], DF16, tag="pmall")

    # ---- DMA schedule: chunk0, lists(t0..t3 between chunks), chunks, passes
    nc.sync.dma_start(y[:, 0], y_d[0])
    for tt in range(NT):
        nc.sync.dma_start(li[tt][:], li_d[tt])
        nc.sync.dma_start(y[:, tt + 1], y_d[tt + 1])
    for c in range(NT + 1, NCH):
        nc.sync.dma_start(y[:, c], y_d[c])
    for tt in range(NT):
        for q in range(1, MAXO + 1):
            nc.sync.dma_start(pq[(tt, q)][:], pq_d[q][tt])
    for tt in range(NT):
        nc.sync.dma_start(pmall[:, tt, :], pm_d[tt])

    # ---- plane generation (gpsimd, early, overlapped with the y8 load)
    for tt in range(NT):
        for qq in range(NQ):
            nc.gpsimd.local_scatter(
                plane[tt][:, qq * QW:(qq + 1) * QW],
                li[tt][:, qq, 1, :], li[tt][:, qq, 0, :],
                128, QW, NEQ)
        # unset cols: 0 - 1 = -1 (ignored); set: (slot+1) - 1 = slot
        nc.vector.tensor_scalar_add(plane[tt][:], plane[tt][:], -1)

    slpool = ctx.enter_context(tc.tile_pool(name="slots", bufs=1))
    dpool = ctx.enter_context(tc.tile_pool(name="dq", bufs=2))
    expool = ctx.enter_context(tc.tile_pool(name="extra", bufs=1))
    elpool = ctx.enter_context(tc.tile_pool(name="elb", bufs=2))

    dq = {}
    with tc.tile_pool(name="gpsum", bufs=1, space="PSUM") as gpsum:
        # ---- chunk-major gram + drains + scatters
        for c in range(NCH):
            for tt in range(NT):
                ps = gpsum.tile([128, CW], F32, tag=f"ps{tt}_{c % 2}")
                for kp in range(NKP):
                    nc.tensor.matmul(
                        ps[:],
                        lhsT=y[:, 0, 2 * kp:2 * kp + 2, tt * 128:(tt + 1) * 128],
                        rhs=y[:, c, 2 * kp:2 * kp + 2, :],
                        start=(kp == 0), stop=(kp == NKP - 1),
                        perf_mode=DR,
                    )
                dst = gbf[tt][:, c * CW:(c + 1) * CW]
                if (c * NT + tt) % 2 == 0:
                    nc.vector.tensor_copy(dst, ps[:])
                else:
                    nc.scalar.copy(dst, ps[:])
            # quarter qq covers chunks 2qq, 2qq+1 -> scatter after odd chunks
            if c % 2 == 1:
                qq = c // 2
                for tt in range(NT):
                    d = dpool.tile([128, NE], DF16, tag=f"d{tt}",
                                   name=f"d{tt}_{qq}")
                    dq[(tt, qq)] = d
                    nc.gpsimd.local_scatter(
                        d[:], gbf[tt][:, qq * QW:(qq + 1) * QW],
                        plane[tt][:, qq * QW:(qq + 1) * QW],
                        128, NE, QW)
                # eager partial sums free the d ring slots
                if c in (3, 7):
                    lo = 0 if c == 3 else 2
                    key = "s01" if c == 3 else "s23"
                    for tt in range(NT):
                        h = slpool.tile([128, NE], DF16, tag=f"s{c // 4}_{tt}",
                                        name=f"h{c // 4}_{tt}")
                        nc.vector.tensor_tensor(h[:], dq[(tt, lo)][:],
                                                dq[(tt, lo + 1)][:], ALU.add)
                        dq[(tt, key)] = h

    # ---- per tile: sum quarters, dup passes, exp, den, l0
    for tt in range(NT):
        s03 = slpool.tile([128, NE], DF16, tag=f"s03_{tt}")
        nc.vector.tensor_tensor(s03[:], dq[(tt, "s01")][:], dq[(tt, "s23")][:],
                                ALU.add)

        eq = {}
        for q in range(1, MAXO + 1):
            e = expool.tile([128, NE], DF16, tag=f"e{q}_{tt % 2}")
            nc.gpsimd.local_scatter(e[:], s03[:, 0:W[q]], pq[(tt, q)][:],
                                    128, NE, W[q])
            eq[q] = e
        s_all = s03
        for q in range(1, MAXO + 1):
            nxt = slpool.tile([128, NE], DF16, tag=f"sa{q}_{tt % 2}")
            nc.vector.tensor_tensor(nxt[:], s_all[:], eq[q][:], ALU.add)
            s_all = nxt

        ebuf = elpool.tile([128, NE], F32, tag="ebuf")
        nc.scalar.activation(ebuf[:], s_all[:], AF.Exp, scale=KSC)
        e3 = ebuf[:, 0:SP * J].rearrange("p (s j) -> p s j", j=J)
        nc.vector.tensor_reduce(denall[:, tt * SP:(tt + 1) * SP], e3,
                                AX.X, ALU.add)
        l0 = s_all[:, 0:SP * J].rearrange("p (s j) -> p s j", j=J)[:, :, 0]
        nc.vector.tensor_copy(l0all[:, tt * SP:(tt + 1) * SP], l0)

    # ---- batched logsumexp tail + total
    with tc.tile_pool(name="p5psum", bufs=1, space="PSUM") as p5psum:
        lnd = lpool.tile([128, NT * SP], F32, tag="lnd")
        nc.scalar.activation(lnd[:], denall[:], AF.Ln)
        diff = lpool.tile([128, NT * SP], F32, tag="diff")
        nc.vector.scalar_tensor_tensor(diff[:], l0all[:], -KSC, lnd[:],
                                       ALU.mult, ALU.add)
        scrap = lpool.tile([128, NT * SP], F32, tag="scrap")
        acc1 = lpool.tile([128, 1], F32, tag="acc1")
        nc.vector.scalar_tensor_tensor(
            scrap[:], diff[:], 1.0,
            pmall[:].rearrange("p t s -> p (t s)"), ALU.mult, ALU.mult,
            accum_out=acc1[:],
        )
        ps = p5psum.tile([1, 1], F32, tag="ps_out")
        nc.tensor.matmul(ps[:], lhsT=acc1[:], rhs=ones_f32c[:, 0:1],
                         start=True, stop=True)
        res = lpool.tile([1, 1], F32, tag="res")
        nc.scalar.copy(res[:], ps[:])
        nc.sync.dma_start(out_d[:], res[:])


def build_nc(SP, NE, NEQ, MAXO, W, enable_asserts=False):
    nc = bacc.Bacc("TRN2", target_bir_lowering=False, debug=False,
                   enable_asserts=enable_asserts, num_devices=NCORES)
    io = {
        "y8": nc.dram_tensor("y8", [NCH, 128, 2 * NKP, CW], F8,
                             kind="ExternalInput").ap(),
        "lists": nc.dram_tensor("lists", [NT, 128, NQ, 2, NEQ], I16,
                                kind="ExternalInput").ap(),
        "pm": nc.dram_tensor("pm", [NT, 128, SP], DF16,
                             kind="ExternalInput").ap(),
        "out": nc.dram_tensor("out", [1, 1], F32, kind="ExternalOutput").ap(),
    }
    for q in range(1, MAXO + 1):
        io[f"pass{q}"] = nc.dram_tensor(
            f"pass{q}", [NT, 128, W[q]], I16, kind="ExternalInput").ap()
    with tile.TileContext(nc) as tc:
        _build(tc, io, SP, NE, NEQ, MAXO, W)
    nc.compile()
    return nc


def make_in_maps(x, plan):
    x = np.asarray(x, np.float32)
    w = np.sqrt((x.astype(np.float64) ** 2).sum(axis=1, keepdims=True))
    w = np.maximum(w, 1e-8)
    xn = (x / w * S).astype(np.float32)
    x8 = np.clip(xn, -240.0, 240.0).astype(FP8)
    in_maps = []
    for k in range(NCORES):
        xr = np.roll(x8, -RPC * k, axis=0)                   # [B, D]
        y8 = xr.T.reshape(2 * NKP, 128, B).transpose(1, 0, 2)  # [128, 16, B]
        y8c = np.ascontiguousarray(
            y8.reshape(128, 2 * NKP, NCH, CW).transpose(2, 0, 1, 3))
        m = {
            "y8": y8c,
            "lists": plan["lists"][k],
            "pm": plan["pairmask"][k],
        }
        for q in range(1, plan["MAXO"] + 1):
            m[f"pass{q}"] = plan["planes"][q][k]
        in_maps.append(m)
    return in_maps


def kernel(**inputs):
    x = np.asarray(inputs["x"], np.float32)
    anchor_idx = np.asarray(inputs["anchor_idx"])
    pos_idx = np.asarray(inputs["pos_idx"])
    neg_idx = np.asarray(inputs["neg_idx"])
    P = anchor_idx.shape[0]

    plan = build_plan(anchor_idx, pos_idx, neg_idx)
    nc = build_nc(plan["SP"], plan["NE"], plan["NEQ"], plan["MAXO"], plan["W"])
    in_maps = make_in_maps(x, plan)
    res = run_bass_kernel_spmd(nc, in_maps, list(range(NCORES)))
    total = sum(float(res.results[k]["out"][0, 0]) for k in range(NCORES))
    return np.float32(total / P)


# revision 11
# speedup vs baseline: 1.7869x; 1.3328x over previous
"""Trainium2 Bass kernel for ContrastiveNet loss (v4: host-normalized rows,
chunk-major gram pipeline, HBM scatter planes, host fixup of rare dups).

Algorithm (per core k of 8, SPMD):
  - host: xn = x / ||x|| * S (S=32), cast fp8e4 (so sim = G / (S^2*TEMP) with
    G the raw fp8 gram; no on-device normalization at all). Rows rolled so
    core k's 512 anchor rows sit at rotated columns 0..511. y laid out
    COLUMN-CHUNK-major: [8 chunks][128 part][16 kt][512 cols] so each chunk's
    gram (all 4 row-tiles x 8 kp DoubleRow matmuls) runs as soon as the chunk
    lands -> gram fully overlaps the HBM load.
  - device, per column chunk c: 4x8 fp8 DR matmuls into [128,512] PSUM tiles
    (8 banks: 4 tiles x double buffer), drained to fp16 SBUF alternately by
    DVE/ACT.
  - gather: per (tile, quarter=1024 cols): gpsimd local_scatter with an HBM
    col->slot plane (int16, -1 = unused), interleaved into the y8 DMA stream
    just-in-time. First duplicate (row,col) refs fixed by one extra scatter
    pass (source occ0 slot -> occ1 slot) reading the summed quarters.
  - loss: exp(scale*logits) on ACT, per-pair den reduce + masked
    (ln den - scale*l0) accumulation, single-partial [1,1] out per core.
  - host: pairs containing occ>=2 refs (~2%) are masked out on device and
    their loss terms computed exactly on host; total = (dev + host) / P.
"""
import os
import sys
import numpy as np
import ml_dtypes

try:
    import concourse  # noqa: F401
except ImportError:
    sys.path.insert(0, "/opt/trn_rl_repo")

from contextlib import ExitStack

import concourse.bass as bass
import concourse.tile as tile
from concourse import bacc, mybir
from concourse._compat import with_exitstack
from concourse.bass_utils import run_bass_kernel_spmd

F16 = np.float16
FP8 = ml_dtypes.float8_e4m3
F32 = mybir.dt.float32
DF16 = mybir.dt.float16
F8 = mybir.dt.float8e4
I16 = mybir.dt.int16

B, D, J = 4096, 2048, 11
NCORES, RPC, NT, NKP = 8, 512, 4, 8
NCH, CW = 8, 512          # column chunks of the gram (per core)
NQ, QW = 4, 1024          # scatter quarters
TEMP = 0.1
S = 32.0                  # host pre-scale of normalized rows
KSC = 1.0 / (S * S * TEMP)
AF = mybir.ActivationFunctionType
ALU = mybir.AluOpType
AX = mybir.AxisListType
DR = mybir.MatmulPerfMode.DoubleRow


def _even(n):
    return n + (n % 2)


# ---------------------------------------------------------------- host prep
def build_plan(anchor_idx, pos_idx, neg_idx):
    r = anchor_idx.astype(np.int64)
    cols = np.concatenate([pos_idx[:, None], neg_idx], axis=1).astype(np.int64)
    P = r.shape[0]

    # ---- duplicate groups over (row, col)
    er = np.repeat(r, J)
    ec = cols.ravel()
    pair_of = np.repeat(np.arange(P), J)
    key = er * B + ec
    o2 = np.argsort(key, kind="stable")
    k_sorted = key[o2]
    first2 = np.r_[True, k_sorted[1:] != k_sorted[:-1]]
    gid_sorted = np.cumsum(first2) - 1
    NG = int(gid_sorted[-1]) + 1
    gid = np.empty(P * J, np.int64)
    gid[o2] = gid_sorted
    occ_sorted = np.arange(P * J) - np.flatnonzero(first2)[gid_sorted]
    occ = np.empty(P * J, np.int64)
    occ[o2] = occ_sorted

    # pairs containing any occ>=2 entry -> host-corrected, masked on device
    bad_pairs = np.unique(pair_of[occ >= 2])
    bad = np.zeros(P, bool)
    bad[bad_pairs] = True

    n_per_row = np.bincount(r, minlength=B)
    SP = int(max(n_per_row.max(), 1))
    NE = _even(SP * J)
    assert NE * 32 < 2**16

    order_p = np.lexsort((np.arange(P), r))
    r_sp = r[order_p]
    firstp = np.r_[True, r_sp[1:] != r_sp[:-1]]
    gidp = np.cumsum(firstp) - 1
    rank_sorted = np.arange(P) - np.flatnonzero(firstp)[gidp]
    srank = np.empty(P, np.int64)
    srank[order_p] = rank_sorted
    eslot = srank[pair_of] * J + np.tile(np.arange(J), P)   # slot per entry

    core = er // RPC
    t = (er % RPC) // 128
    pp = er % 128
    ec_rot = (ec - core * RPC) % B

    # ---- main scatter plane: col -> occ0 slot (quarter-split), -1 = unused
    m0 = occ == 0
    plane = np.full((NCORES, NT, NQ, 128, QW), -1, np.int16)
    plane[core[m0], t[m0], ec_rot[m0] // QW, pp[m0], ec_rot[m0] % QW] = \
        eslot[m0].astype(np.int16)

    # ---- pass-1 plane: source occ0 slot -> occ1 slot
    slot0_g = np.empty(NG, np.int64)
    slot0_g[gid[m0]] = eslot[m0]
    m1 = occ == 1
    planeA = np.full((NCORES, NT, 128, NE), -1, np.int16)
    planeA[core[m1], t[m1], pp[m1], slot0_g[gid[m1]]] = eslot[m1].astype(np.int16)
    have_pass = bool(m1.any())

    nmat = n_per_row.reshape(NCORES, NT, 128)
    pairmask = (np.arange(SP)[None, None, None, :] < nmat[..., None]).astype(F16)
    # zero out host-corrected pairs
    bp = bad_pairs
    pairmask[r[bp] // RPC, (r[bp] % RPC) // 128, r[bp] % 128, srank[bp]] = 0

    return dict(plane=plane, planeA=planeA, pairmask=pairmask,
                SP=SP, NE=NE, have_pass=have_pass, bad_pairs=bad_pairs)


# ------------------------------------------------------------- device kernel
@with_exitstack
def _build(ctx: ExitStack, tc: "tile.TileContext", io: dict, SP: int, NE: int,
           have_pass: bool):
    nc = tc.nc
    y_d, pl_d, pm_d, out_d = io["y8"], io["plane"], io["pm"], io["out"]
    pa_d = io.get("passA")

    consts = ctx.enter_context(tc.tile_pool(name="consts", bufs=1))
    ones_f32c = consts.tile([128, 1], F32, tag="ones_f32c")
    nc.vector.memset(ones_f32c[:], 1.0)

    ypool = ctx.enter_context(tc.tile_pool(name="y", bufs=1))
    y = ypool.tile([128, NCH, 2 * NKP, CW], F8, tag="y", name="y")

    gpool = ctx.enter_context(tc.tile_pool(name="gbf", bufs=1))
    gbf = {tt: gpool.tile([128, B], DF16, tag=f"gbf{tt}", name=f"gbf{tt}")
           for tt in range(NT)}
    plpool = ctx.enter_context(tc.tile_pool(name="plane", bufs=2))
    papool = ctx.enter_context(tc.tile_pool(name="passA", bufs=1))
    pa = {}
    if have_pass:
        pa = {tt: papool.tile([128, NE], I16, tag=f"pa{tt}", name=f"pa{tt}")
              for tt in range(NT)}

    lpool = ctx.enter_context(tc.tile_pool(name="loss", bufs=1))
    denall = lpool.tile([128, NT * SP], F32, tag="denall")
    l0all = lpool.tile([128, NT * SP], DF16, tag="l0all")
    pmall = lpool.tile([128, NT, SP], DF16, tag="pmall")

    # ---- DMA: chunks with plane quarters interleaved just-in-time
    pl = {}
    def load_plane_level(qq):
        for tt in range(NT):
            p = plpool.tile([128, QW], I16, tag=f"pl{tt}", name=f"pl{tt}_{qq}")
            pl[(tt, qq)] = p
            nc.sync.dma_start(p[:], pl_d[tt, qq])

    nc.sync.dma_start(y[:, 0], y_d[0])
    nc.sync.dma_start(y[:, 1], y_d[1])
    load_plane_level(0)
    nc.sync.dma_start(y[:, 2], y_d[2])
    nc.sync.dma_start(y[:, 3], y_d[3])
    load_plane_level(1)
    nc.sync.dma_start(y[:, 4], y_d[4])
    nc.sync.dma_start(y[:, 5], y_d[5])
    load_plane_level(2)
    nc.sync.dma_start(y[:, 6], y_d[6])
    nc.sync.dma_start(y[:, 7], y_d[7])
    load_plane_level(3)
    if have_pass:
        for tt in range(NT):
            nc.sync.dma_start(pa[tt][:], pa_d[tt])
    for tt in range(NT):
        nc.sync.dma_start(pmall[:, tt, :], pm_d[tt])

    slpool = ctx.enter_context(tc.tile_pool(name="slots", bufs=1))
    dpool = ctx.enter_context(tc.tile_pool(name="dq", bufs=2))
    expool = ctx.enter_context(tc.tile_pool(name="extra", bufs=2))
    elpool = ctx.enter_context(tc.tile_pool(name="elb", bufs=2))

    dq = {}
    with tc.tile_pool(name="gpsum", bufs=1, space="PSUM") as gpsum:
        # ---- chunk-major gram + drains + scatters
        for c in range(NCH):
            for tt in range(NT):
                ps = gpsum.tile([128, CW], F32, tag=f"ps{tt}_{c % 2}",
                                name=f"ps{tt}_{c}")
                for kp in range(NKP):
                    nc.tensor.matmul(
                        ps[:],
                        lhsT=y[:, 0, 2 * kp:2 * kp + 2, tt * 128:(tt + 1) * 128],
                        rhs=y[:, c, 2 * kp:2 * kp + 2, :],
                        start=(kp == 0), stop=(kp == NKP - 1),
                        perf_mode=DR,
                    )
                dst = gbf[tt][:, c * CW:(c + 1) * CW]
                if (c * NT + tt) % 2 == 0:
                    nc.vector.tensor_copy(dst, ps[:])
                else:
                    nc.scalar.copy(dst, ps[:])
            # quarter qq covers chunks 2qq, 2qq+1 -> scatter after odd chunks
            if c % 2 == 1:
                qq = c // 2
                for tt in range(NT):
                    d = dpool.tile([128, NE], DF16, tag=f"d{tt}",
                                   name=f"d{tt}_{qq}")
                    dq[(tt, qq)] = d
                    nc.gpsimd.local_scatter(
                        d[:], gbf[tt][:, qq * QW:(qq + 1) * QW],
                        pl[(tt, qq)][:], 128, NE, QW)
                # eager partial sums free the d ring slots
                if c in (3, 7):
                    lo = 0 if c == 3 else 2
                    key = "s01" if c == 3 else "s23"
                    for tt in range(NT):
                        h = slpool.tile([128, NE], DF16, tag=f"h{c // 4}_{tt}",
                                        name=f"h{c // 4}_{tt}")
                        nc.vector.tensor_tensor(h[:], dq[(tt, lo)][:],
                                                dq[(tt, lo + 1)][:], ALU.add)
                        dq[(tt, key)] = h

    # ---- per tile: sum halves, dup pass, exp, den, l0
    for tt in range(NT):
        s03 = slpool.tile([128, NE], DF16, tag=f"s03_{tt}")
        nc.vector.tensor_tensor(s03[:], dq[(tt, "s01")][:], dq[(tt, "s23")][:],
                                ALU.add)
        if have_pass:
            eA = expool.tile([128, NE], DF16, tag=f"eA{tt % 2}",
                             name=f"eA{tt}")
            nc.gpsimd.local_scatter(eA[:], s03[:], pa[tt][:], 128, NE, NE)
            s_all = slpool.tile([128, NE], DF16, tag=f"sa{tt % 2}",
                                name=f"sa{tt}")
            nc.vector.tensor_tensor(s_all[:], s03[:], eA[:], ALU.add)
        else:
            s_all = s03

        ebuf = elpool.tile([128, NE], F32, tag="ebuf")
        nc.scalar.activation(ebuf[:], s_all[:], AF.Exp, scale=KSC)
        e3 = ebuf[:, 0:SP * J].rearrange("p (s j) -> p s j", j=J)
        nc.vector.tensor_reduce(denall[:, tt * SP:(tt + 1) * SP], e3,
                                AX.X, ALU.add)
        l0 = s_all[:, 0:SP * J].rearrange("p (s j) -> p s j", j=J)[:, :, 0]
        nc.vector.tensor_copy(l0all[:, tt * SP:(tt + 1) * SP], l0)

    # ---- batched logsumexp tail + total
    with tc.tile_pool(name="p5psum", bufs=1, space="PSUM") as p5psum:
        lnd = lpool.tile([128, NT * SP], F32, tag="lnd")
        nc.scalar.activation(lnd[:], denall[:], AF.Ln)
        diff = lpool.tile([128, NT * SP], F32, tag="diff")
        nc.vector.scalar_tensor_tensor(diff[:], l0all[:], -KSC, lnd[:],
                                       ALU.mult, ALU.add)
        scrap = lpool.tile([128, NT * SP], F32, tag="scrap")
        acc1 = lpool.tile([128, 1], F32, tag="acc1")
        nc.vector.scalar_tensor_tensor(
            scrap[:], diff[:], 1.0,
            pmall[:].rearrange("p t s -> p (t s)"), ALU.mult, ALU.mult,
            accum_out=acc1[:],
        )
        ps = p5psum.tile([1, 1], F32, tag="ps_out")
        nc.tensor.matmul(ps[:], lhsT=acc1[:], rhs=ones_f32c[:, 0:1],
                         start=True, stop=True)
        res = lpool.tile([1, 1], F32, tag="res")
        nc.scalar.copy(res[:], ps[:])
        nc.sync.dma_start(out_d[:], res[:])


def build_nc(SP, NE, have_pass, enable_asserts=False):
    nc = bacc.Bacc("TRN2", target_bir_lowering=False, debug=False,
                   enable_asserts=enable_asserts, num_devices=NCORES)
    io = {
        "y8": nc.dram_tensor("y8", [NCH, 128, 2 * NKP, CW], F8,
                             kind="ExternalInput").ap(),
        "plane": nc.dram_tensor("plane", [NT, NQ, 128, QW], I16,
                                kind="ExternalInput").ap(),
        "pm": nc.dram_tensor("pm", [NT, 128, SP], DF16,
                             kind="ExternalInput").ap(),
        "out": nc.dram_tensor("out", [1, 1], F32, kind="ExternalOutput").ap(),
    }
    if have_pass:
        io["passA"] = nc.dram_tensor("passA", [NT, 128, NE], I16,
                                     kind="ExternalInput").ap()
    with tile.TileContext(nc) as tc:
        _build(tc, io, SP, NE, have_pass)
    nc.compile()
    return nc


def _normalize(x):
    x = np.asarray(x, np.float32)
    w = np.sqrt((x.astype(np.float64) ** 2).sum(axis=1, keepdims=True))
    w = np.maximum(w, 1e-8)
    return (x / w).astype(np.float32)


def make_in_maps(x, plan):
    xn = _normalize(x)
    x8 = np.clip(xn * S, -240.0, 240.0).astype(FP8)
    in_maps = []
    for k in range(NCORES):
        xr = np.roll(x8, -RPC * k, axis=0)                     # [B, D]
        y8 = xr.T.reshape(2 * NKP, 128, B).transpose(1, 0, 2)  # [128, 16, B]
        y8c = np.ascontiguousarray(
            y8.reshape(128, 2 * NKP, NCH, CW).transpose(2, 0, 1, 3))
        m = {
            "y8": y8c,
            "plane": plan["plane"][k],
            "pm": plan["pairmask"][k],
        }
        if plan["have_pass"]:
            m["passA"] = plan["planeA"][k]
        in_maps.append(m)
    return in_maps


def host_fixup(x, anchor_idx, pos_idx, neg_idx, bad_pairs):
    """Exact loss terms for pairs masked out on the device."""
    if len(bad_pairs) == 0:
        return 0.0
    xn = _normalize(x).astype(np.float64)
    a = anchor_idx[bad_pairs]
    cols = np.concatenate([pos_idx[bad_pairs][:, None], neg_idx[bad_pairs]],
                          axis=1)
    logits = np.einsum("pd,pjd->pj", xn[a], xn[cols]) / TEMP
    mx = logits.max(axis=1, keepdims=True)
    lse = np.log(np.exp(logits - mx).sum(axis=1)) + mx[:, 0]
    return float((lse - logits[:, 0]).sum())


def kernel(**inputs):
    x = np.asarray(inputs["x"], np.float32)
    anchor_idx = np.asarray(inputs["anchor_idx"])
    pos_idx = np.asarray(inputs["pos_idx"])
    neg_idx = np.asarray(inputs["neg_idx"])
    P = anchor_idx.shape[0]

    plan = build_plan(anchor_idx, pos_idx, neg_idx)
    nc = build_nc(plan["SP"], plan["NE"], plan["have_pass"])
    in_maps = make_in_maps(x, plan)
    res = run_bass_kernel_spmd(nc, in_maps, list(range(NCORES)))
    total = sum(float(res.results[k]["out"][0, 0]) for k in range(NCORES))
    total += host_fixup(x, anchor_idx, pos_idx, neg_idx, plan["bad_pairs"])
    return np.float32(total / P)
